# revision 14
# baseline (speedup 1.0000x reference)
"""Trainium2 Bass kernel for nn_AttentionHead_86715389706346.

Mathematical background
-----------------------
The reference module computes, per batch b (x: [T, C]):
    q = x @ Wq ; k = x @ Wk ; v = x @ Wv
    attn = (q @ k.T) / sqrt(d)                       [T, T]
    attn = attn @ mask          (mask is all ones)
    p    = softmax(attn, axis=0)  (over the query axis)
    out  = p @ v

Because mask is the all-ones matrix, (attn @ mask)[q, t] = sum_k attn[q, k]
is independent of t, and the softmax over the query axis of a
column-constant matrix is column-constant, so the output collapses to a
rank-1 outer product:

    s[t]  = q[t, :] . ksum,    ksum = Wk^T xsum,  xsum = sum_t x[t, :]
    out   = softmax(alpha*s) (x) vsum,   vsum = Wv^T xsum

Kernel structure (per core = per batch)
---------------------------------------
The host pre-transposes x to fp16 xT[c, t] stored as [p, j, t] (c = 128j+p)
and pre-permutes the fp16 weights to [p, j, d] (c = 128j+p).  fp16 halves
DMA bytes; the rel-err budget (2e-2) holds with ~9x margin (verified in
fp64 simulation against the reference: 2.3e-3).

  - Weights ride the two HWDGE rings (sync/scalar engines) FIRST (small,
    and the first q matmul needs Wq); x follows as 4 x 1MB chunk-pair
    DMAs alternating between the rings (8 KB descriptors).
  - As each chunk lands: PE accumulates qT[d, t] += Wq_j^T xT_j into 4
    PSUM banks; xsum_j is reduced in two halves (scalar activation
    accum_out + vector affine_mul_reduce against a ones tile, both
    engines otherwise idle); per-chunk ksum/vsum matmuls accumulate the
    half-partials (2-col rhs) so only a tiny fold remains at the end.
  - Tail: per-bank qT->SBUF fp16 copies interleaved with the s matmuls
    (s[t] = qT_block^T ksum, 16 stationary-qT matmuls -> s[p, i] with
    t = 128 i + p), global-max softmax (alpha*s spans ~ +-200 so exp
    needs the max), vsum broadcast row, 16 scaled copies for the rank-1
    output, DMA'd out in two halves on the two rings.

Distribution: data-parallel over batch; B == 8 == number of NeuronCores.
"""

import numpy as np

T = 2048
IN_C = 1024
D = 128
P = 128
NC = IN_C // P   # 8 channel chunks
NT = T // P      # 16 token tiles
B = 8
ALPHA = float(1.0 / np.sqrt(128.0))

_NC_CACHE = {}


def build_bass():
    import concourse.bass as bass
    import concourse.bacc as bacc
    import concourse.mybir as mybir
    import concourse.tile as tile
    from concourse.masks import make_identity

    f32 = mybir.dt.float32
    f16 = mybir.dt.float16
    AF = mybir.ActivationFunctionType
    OP = mybir.AluOpType

    nc = bacc.Bacc()
    # host-pretransposed x: [p, j, t] = x[t, 128j+p], fp16
    x_ext = nc.declare_dram_parameter("xT", [P, NC, T], f16, isOutput=False)
    # host-prepermuted weights: [p, j, d] = W[128j+p, d], fp16
    wq_ext = nc.declare_dram_parameter("Wq", [P, NC, D], f16, isOutput=False)
    wk_ext = nc.declare_dram_parameter("Wk", [P, NC, D], f16, isOutput=False)
    wv_ext = nc.declare_dram_parameter("Wv", [P, NC, D], f16, isOutput=False)
    # out[p, i, d] = out[t = 128i+p, d], fp16 (host reassembles)
    out_ext = nc.declare_dram_parameter("out", [P, NT, D], f16, isOutput=True)

    with tile.TileContext(nc) as tc:
        with (
            tc.tile_pool(name="const", bufs=1) as cpool,
            tc.tile_pool(name="xbuf", bufs=1) as xbuf,
            tc.tile_pool(name="wbuf", bufs=1) as wbuf,
            tc.tile_pool(name="work", bufs=1) as work,
            tc.tile_pool(name="scr", bufs=2) as scr,
            tc.tile_pool(name="pq", bufs=1, space="PSUM") as pqp,
            tc.tile_pool(name="psm", bufs=1, space="PSUM") as psmp,
            tc.tile_pool(name="pvs", bufs=1, space="PSUM") as pvsp,
            tc.tile_pool(name="prow", bufs=1, space="PSUM") as prowp,
        ):
            # ---- weights first on the two HWDGE rings (q needs Wq early) --
            wq_sb = wbuf.tile([P, NC, D], f16)
            nc.sync.dma_start(out=wq_sb, in_=wq_ext[:, :, :])
            wk_sb = wbuf.tile([P, NC, D], f16)
            nc.scalar.dma_start(out=wk_sb, in_=wk_ext[:, :, :])
            wv_sb = wbuf.tile([P, NC, D], f16)
            nc.scalar.dma_start(out=wv_sb, in_=wv_ext[:, :, :])

            # ---- x: ring A gets j0-3, ring B gets j4-7 (the sync ring
            # drains first in practice, so arrival stays monotone in j) ----
            xT = xbuf.tile([P, NC, T], f16, tag="xT")
            nc.sync.dma_start(out=xT[:, 0:2, :], in_=x_ext[:, 0:2, :])
            nc.sync.dma_start(out=xT[:, 2:4, :], in_=x_ext[:, 2:4, :])
            nc.scalar.dma_start(out=xT[:, 4:6, :], in_=x_ext[:, 4:6, :])
            nc.scalar.dma_start(out=xT[:, 6:8, :], in_=x_ext[:, 6:8, :])

            # ---- constants ----
            ident = cpool.tile([P, P], f32)
            make_identity(nc, ident)
            ones_col = cpool.tile([P, 1], f32)
            nc.vector.memset(ones_col, 1.0)
            ones_row = cpool.tile([1, P], f32)
            nc.vector.memset(ones_row, 1.0)
            ones16 = cpool.tile([P, T // 2], f16)
            nc.vector.memset(ones16, 1.0)

            # preload exp table off the critical path
            dummy = work.tile([P, 1], f32, tag="dummy")
            nc.scalar.activation(out=dummy, in_=ones_col, func=AF.Exp)

            # PSUM layout
            q_ps = pqp.tile([P, 4 * 512], f32, tag="q")  # 4 banks, qT [d, t]
            small = psmp.tile([P, 512], f32, tag="small")
            ks2_ps = small[:, 0:2]
            # vsum accumulates in its own bank: a start=True matmul clears
            # has_written for the WHOLE bank, so two concurrently-open
            # accumulation groups must not share one.
            vs2_ps = pvsp.tile([P, 2], f32, tag="vs2")
            s_ps = small[:, 16:32]
            pnm = small[:, 32:33]
            pr = small[:, 33:34]
            pvbc = small[:, 64:192]
            row = prowp.tile([1, 512], f32, tag="row")
            pm = row[:, 0:128]
            pS = row[:, 128:129]
            pvT = row[:, 256:384]

            # warm the PE clock (1.2 GHz cold -> 2.4 GHz after ~4us of
            # sustained work) with throwaway matmuls gated only on the
            # ones16 memset, so it is hot when the first x chunk lands
            warm_ps = pvsp.tile([P, 504], f32, tag="warm")
            for _ in range(8):
                nc.tensor.matmul(warm_ps, lhsT=ones16[:, 0:128],
                                 rhs=ones16[:, 0:504], start=True, stop=True)

            # ---- streaming phase, per chunk j ----
            xs2 = work.tile([P, 2 * NC], f32, tag="xs2")    # half-partials
            xs16 = work.tile([P, 2 * NC], f16, tag="xs16")
            HA = 1068            # ACT share (1.2 GHz) vs DVE share (0.96)
            for j in range(NC):
                # q: Wq_j stationary, xT_j streaming into 4 PSUM banks
                for tb in range(4):
                    nc.tensor.matmul(q_ps[:, 512 * tb:512 * (tb + 1)],
                                     lhsT=wq_sb[:, j, :],
                                     rhs=xT[:, j, 512 * tb:512 * (tb + 1)],
                                     start=(j == 0), stop=(j == NC - 1))
                # xsum_j in two halves: scalar engine + vector engine
                zA = scr.tile([P, HA], f16, tag="zA")
                nc.scalar.activation(out=zA, in_=xT[:, j, 0:HA], func=AF.Copy,
                                     accum_out=xs2[:, 2 * j:2 * j + 1])
                zB = scr.tile([P, T - HA], f16, tag="zB")
                nc.vector.affine_mul_reduce(
                    out=zB, accum_out=xs2[:, 2 * j + 1:2 * j + 2],
                    in0=xT[:, j, HA:T], in1=ones16[:, 0:T - HA],
                    scale=1.0, bias=0.0)
                nc.vector.tensor_copy(out=xs16[:, 2 * j:2 * j + 2],
                                      in_=xs2[:, 2 * j:2 * j + 2])

            # ksum/vsum after the q loop: a stalled matmul in the stream
            # loop would block the later q matmuls in the PE FIFO
            for j in range(NC):
                nc.tensor.matmul(ks2_ps, lhsT=wk_sb[:, j, :],
                                 rhs=xs16[:, 2 * j:2 * j + 2],
                                 start=(j == 0), stop=(j == NC - 1))
            for j in range(NC):
                nc.tensor.matmul(vs2_ps, lhsT=wv_sb[:, j, :],
                                 rhs=xs16[:, 2 * j:2 * j + 2],
                                 start=(j == 0), stop=(j == NC - 1))

            # ---- vsum fold early (needs only vs2) ----
            vsum_sb = work.tile([P, 1], f32, tag="vsum_sb")
            nc.vector.reduce_sum(out=vsum_sb, in_=vs2_ps,
                                 axis=mybir.AxisListType.X)
            nc.tensor.transpose(pvT, vsum_sb, ident)

            # ---- fold ksum halves -> fp16 column ----
            ksum16 = work.tile([P, 1], f16, tag="ksum16")
            with nc.allow_low_precision(reason="2-term fold; fp16 quant modeled"):
                nc.vector.reduce_sum(out=ksum16, in_=ks2_ps,
                                     axis=mybir.AxisListType.X)

            # ---- qT -> SBUF fp16 per bank, interleaved with s matmuls ----
            qT16 = work.tile([P, T], f16, tag="qT16")
            for tb in range(4):
                sl = slice(512 * tb, 512 * (tb + 1))
                if tb % 2 == 0:
                    nc.scalar.activation(out=qT16[:, sl], in_=q_ps[:, sl],
                                         func=AF.Copy)
                else:
                    nc.vector.tensor_copy(out=qT16[:, sl], in_=q_ps[:, sl])
                for i in range(4 * tb, 4 * tb + 4):
                    nc.tensor.matmul(s_ps[:, i:i + 1],
                                     lhsT=qT16[:, P * i:P * (i + 1)],
                                     rhs=ksum16, start=True, stop=True)

            # vsum broadcast row (vrow/pvbc emitted after the s matmuls so
            # the scheduler does not let them block the PE / ACT queues)
            vrow = work.tile([1, P], f32, tag="vrow")
            nc.scalar.activation(out=vrow, in_=pvT, func=AF.Copy)
            nc.tensor.matmul(pvbc, lhsT=ones_row, rhs=vrow, start=True,
                             stop=True)

            # ---- softmax with global max (alpha*s spans ~ +-200) ----
            m1 = work.tile([P, 1], f32, tag="m1")
            nc.vector.reduce_max(out=m1, in_=s_ps, axis=mybir.AxisListType.X)
            nc.tensor.transpose(pm, m1, ident)
            negm_s = work.tile([1, 1], f32, tag="negm_s")
            nc.vector.reduce_max(out=negm_s, in_=pm, axis=mybir.AxisListType.X,
                                 negate=True)
            nc.tensor.matmul(pnm, lhsT=ones_row, rhs=negm_s, start=True,
                             stop=True)
            negam = work.tile([P, 1], f32, tag="negam")
            nc.vector.tensor_scalar(out=negam, in0=pnm, scalar1=ALPHA,
                                    scalar2=None, op0=OP.mult)
            e_sb = work.tile([P, NT], f32, tag="e_sb")
            esum = work.tile([P, 1], f32, tag="esum")
            nc.scalar.activation(out=e_sb, in_=s_ps, func=AF.Exp, bias=negam,
                                 scale=ALPHA, accum_out=esum)

            # 1/sum(e), folded into the broadcast vsum row:
            # out[t, d] = e[t] * (r * vsum[d])
            nc.tensor.matmul(pS, lhsT=esum, rhs=ones_col, start=True, stop=True)
            r_s = work.tile([1, 1], f32, tag="r_s")
            nc.vector.reciprocal(out=r_s, in_=pS)
            nc.tensor.matmul(pr, lhsT=ones_row, rhs=r_s, start=True, stop=True)
            r_bc = work.tile([P, 1], f32, tag="r_bc")
            nc.vector.tensor_copy(out=r_bc, in_=pr)
            vbc16 = work.tile([P, P], f16, tag="vbc16")
            nc.vector.tensor_scalar(out=vbc16, in0=pvbc, scalar1=r_bc,
                                    scalar2=None, op0=OP.mult)

            # ---- out[t, d] = er[t] * vsum[d]; two DMA halves ----
            out_sb = xbuf.tile([P, NT, D], f16, tag="out_sb")
            for i in range(NT):
                if i % 4 == 3:
                    nc.scalar.activation(out=out_sb[:, i, :], in_=vbc16,
                                         func=AF.Copy, scale=e_sb[:, i:i + 1])
                else:
                    nc.vector.tensor_scalar(out=out_sb[:, i, :], in0=vbc16,
                                            scalar1=e_sb[:, i:i + 1],
                                            scalar2=None, op0=OP.mult)
                if i == 7:
                    nc.sync.dma_start(out=out_ext[:, 0:8, :],
                                      in_=out_sb[:, 0:8, :])
            nc.scalar.dma_start(out=out_ext[:, 8:16, :], in_=out_sb[:, 8:16, :])

    nc.finalize()
    return nc


def _get_nc():
    if "nc" not in _NC_CACHE:
        _NC_CACHE["nc"] = build_bass()
    return _NC_CACHE["nc"]


def _prep_host(inputs):
    f16 = np.float16
    x = np.asarray(inputs["x"], dtype=np.float32)
    assert x.shape == (B, T, IN_C)
    # xT[b, p, j, t] = x[b, t, 128j+p]
    xT = np.ascontiguousarray(
        x.astype(f16).transpose(0, 2, 1).reshape(B, NC, P, T).transpose(0, 2, 1, 3)
    )
    ws = []
    for k in ("Wq", "Wk", "Wv"):
        w = np.asarray(inputs[k], dtype=np.float32).astype(f16)
        ws.append(np.ascontiguousarray(
            w.reshape(NC, P, D).transpose(1, 0, 2)))
    return xT, ws


def run(inputs, trace=False, **kwargs):
    """Run on 8 NeuronCores; returns (output [8, 2048, 128], BassKernelResults)."""
    from concourse.bass_utils import run_bass_kernel_spmd

    xT, (wq, wk, wv) = _prep_host(inputs)
    nc = _get_nc()
    in_maps = [
        {"xT": np.ascontiguousarray(xT[i]), "Wq": wq, "Wk": wk, "Wv": wv}
        for i in range(B)
    ]
    res = run_bass_kernel_spmd(nc, in_maps, core_ids=list(range(B)), trace=trace,
                               **kwargs)
    # out[p, i, d] -> [t = 128 i + p, d]
    out = np.stack(
        [np.asarray(res.results[i]["out"]).transpose(1, 0, 2).reshape(T, D)
         for i in range(B)], axis=0)
    return out.astype(np.float32), res


def kernel(**inputs) -> np.ndarray:
    out, _ = run(inputs, trace=False)
    return out
